# revision 28
# baseline (speedup 1.0000x reference)
"""Trainium2 Bass kernel for nn_DiagOpLayer (CG solve on masked tridiagonal op).

Math: reference runs MAX_ITER=20 CG iterations on K = G^T (D^T W^2 D) G where
G scatters the n_miss unknowns into the full n-length signal and D is the
first-difference operator.  We run the CG in the FULL space on masked vectors:
all CG iterates stay exactly supported on the unknown-mask, so
  K_full(p) = unk * D^T(w^2 * D p)
reproduces the compressed iteration exactly (off-mask entries stay 0.0).

Layout (rows-fused): both batch rows of a core live in ONE [128, WF] tile set:
row = partition//64, and partition p holds global positions
[4096*(p%64) - H, 4096*(p%64) + 4095 + H] of that row (H-halo each side).
All ops are elementwise or +-1 shifts along the free dim, so halo validity
shrinks by <=1 column per side per CG iteration and no cross-partition traffic
is needed inside the loop (H=24 > 20 iterations + pre/post shifts).  Fusing
rows halves instruction count (per-instruction dispatch/sem overhead was a
large fraction of runtime) and per-row scalars stay per-partition.

Per-row dot products: ACT square-accumulate -> [128,1] partials -> one PE
matmul against a [128,2] row-block indicator -> [2,1] PSUM; the alpha/beta
chain runs on [2,1] tiles; broadcast back to [128,1] via a [2,128] indicator
matmul.

Sharding: data-parallel, 2 batch rows per core across 8 cores; unknown-mask
replicated.
"""

import sys

import numpy as np

for _p in ("/opt/trn_rl_repo",):
    if _p not in sys.path:
        sys.path.insert(0, _p)

# Problem constants (hardcoded per spec; kernel.py must be self-contained).
B = 16
N = 262144
M = N - 1
NMISS = 131072
MAX_ITER = 20
EPS_W = 1e-3
CLAMP_MIN = 1e-4
CLAMP_MAX = 1e4
NCORES = 8
ROWS = B // NCORES   # 2 batch rows per core
PPR = 128 // ROWS    # partitions per row
CH = N // PPR        # 4096 core columns per partition
H = 24               # halo columns each side (> 20 iters + pre/post shifts)
WF = CH + 2 * H
PADN = N + 2 * H     # padded HBM row length so the halo'd DMA AP stays in-bounds
PAD_U = -60.0        # softplus(-60) ~ 9e-27 -> padded w = EPS_W; contributes
                     # O(w^2)=1e-6-weighted terms only at out-of-range edges,
                     # which are multiplied by exact-0 mask/pad values or are
                     # ~1e-12 relative -- far below fp32 noise.
FSPLIT = 0.44        # column fraction of the big tensor_tensor ops run on DVE
                     # (rest on GPSIMD, which has no other work in the loop)
FSTT = 0.61          # DVE column fraction of the stt updates (GPSIMD emulates
                     # stt as mul+add, so its share costs 2 passes)

_CACHE = {}


def _build_program(debug=False, enable_asserts=False, repeat=1):
    """Build the SPMD Bass/Tile program for one core (2 batch rows).

    repeat>1 wraps the whole body in a hardware For_i loop (timing only:
    difference two repeat values to cancel RPC/launch overhead).
    """
    from contextlib import ExitStack, nullcontext

    import concourse.bass as bass
    import concourse.tile as tile
    from concourse import bacc, mybir

    f32 = mybir.dt.float32
    Alu = mybir.AluOpType
    Act = mybir.ActivationFunctionType

    nc = bacc.Bacc(
        "TRN2",
        target_bir_lowering=False,
        debug=debug,
        enable_asserts=enable_asserts,
    )

    u_in = nc.dram_tensor("u_in", [ROWS, PADN], f32, kind="ExternalInput")
    x_in = nc.dram_tensor("x_in", [ROWS, PADN], f32, kind="ExternalInput")
    unk_in = nc.dram_tensor("unk_in", [PADN], f32, kind="ExternalInput")

    v_out = nc.dram_tensor("v_out", [ROWS, N], f32, kind="ExternalOutput")
    r_out = nc.dram_tensor("r_out", [ROWS, N], f32, kind="ExternalOutput")
    w_out = nc.dram_tensor("w_out", [ROWS, N], f32, kind="ExternalOutput")
    phi_out = nc.dram_tensor("phi_out", [ROWS, 1], f32, kind="ExternalOutput")

    CORE = slice(H, H + CH)

    with tile.TileContext(nc) as tc, ExitStack() as ctx:
        big = ctx.enter_context(tc.tile_pool(name="big", bufs=1))
        tiny = ctx.enter_context(tc.tile_pool(name="tiny", bufs=1))
        parts = ctx.enter_context(tc.tile_pool(name="parts", bufs=4))
        # PSUM has 8 banks; each tile takes a full bank.
        psum = ctx.enter_context(
            tc.tile_pool(name="psum", bufs=4, space=bass.MemorySpace.PSUM)
        )

        # --- persistent tiles ---
        unk_t = big.tile([128, WF], f32, tag="unk")
        u_t = big.tile([128, WF], f32, tag="u")      # u logits -> Kp scratch
        x_t = big.tile([128, WF], f32, tag="x")      # x -> xk -> ACT junk
        w_t = big.tile([128, WF], f32, tag="w")
        b_t = big.tile([128, WF], f32, tag="b")
        V_t = big.tile([128, WF], f32, tag="V")
        rc_t = big.tile([128, WF], f32, tag="rc")
        p_a = big.tile([128, WF], f32, tag="p_a")
        p_b = big.tile([128, WF], f32, tag="p_b")
        s1_t = big.tile([128, WF], f32, tag="s1")
        s2_t = big.tile([128, WF], f32, tag="s2")

        # row-block indicator matrices for per-row dot/broadcast matmuls
        browsT = tiny.tile([128, ROWS], f32, tag="browsT")   # [p, r] = (row(p)==r)
        brows = tiny.tile([ROWS, 128], f32, tag="brows")     # transpose of above
        rs_a = tiny.tile([ROWS, 1], f32, tag="rs_a")
        rs_b = tiny.tile([ROWS, 1], f32, tag="rs_b")
        den = tiny.tile([ROWS, 1], f32, tag="den")
        alpha = tiny.tile([ROWS, 1], f32, tag="alpha")
        beta = tiny.tile([ROWS, 1], f32, tag="beta")
        phi_t = tiny.tile([ROWS, 1], f32, tag="phi")
        nabc = tiny.tile([128, 1], f32, tag="nabc")
        abc_sb = tiny.tile([128, 1], f32, tag="abc_sb")
        bbc_sb = tiny.tile([128, 1], f32, tag="bbc_sb")
        t3s = tiny.tile([ROWS, 1], f32, tag="t3s")
        aas = tiny.tile([ROWS, 1], f32, tag="aas")

        browsT_np = np.zeros((128, ROWS), np.float32)
        for r in range(ROWS):
            browsT_np[r * PPR:(r + 1) * PPR, r] = 1.0
        browsT_dr = nc.inline_tensor(browsT_np, name="browsT_c")
        brows_dr = nc.inline_tensor(browsT_np.T.copy(), name="brows_c")
        nc.sync.dma_start(out=browsT[:], in_=browsT_dr[:])
        nc.sync.dma_start(out=brows[:], in_=brows_dr[:])

        def row_dma(dram, r, tile_ap, width, load):
            ap = bass.AP(dram, r * (PADN if width == WF else N), [[CH, PPR], [1, width]])
            sb = tile_ap[r * PPR:(r + 1) * PPR, 0:width]
            if load:
                nc.sync.dma_start(out=sb, in_=ap)
            else:
                nc.sync.dma_start(out=ap, in_=sb)

        def rowdot(src_core_ap, ps_tag="ps_s"):
            """sum over free dim per partition (ACT) then per row-block (PE)."""
            part = parts.tile([128, 1], f32, tag="part")
            nc.scalar.activation(
                out=x_t[:, 0:CH], in_=src_core_ap, func=Act.Square,
                accum_out=part[:],
            )
            ps = psum.tile([ROWS, 1], f32, tag=ps_tag)
            nc.tensor.matmul(ps[:], browsT[:], part[:], start=True, stop=True)
            return ps

        loop_cm = tc.For_i(0, repeat, 1) if repeat > 1 else nullcontext()
        loop_cm.__enter__()

        # ---------- preamble ----------
        for r in range(ROWS):
            row_dma(u_in, r, u_t, WF, load=True)
            row_dma(x_in, r, x_t, WF, load=True)
            # unknown mask: same for both row blocks
            nc.sync.dma_start(
                out=unk_t[r * PPR:(r + 1) * PPR, :],
                in_=bass.AP(unk_in, 0, [[CH, PPR], [1, WF]]),
            )
        nc.gpsimd.memset(V_t[:], 0.0)
        nc.gpsimd.memset(p_a[:], 0.0)
        nc.gpsimd.memset(p_b[:], 0.0)
        nc.gpsimd.memset(rc_t[:], 0.0)

        # w = clip(softplus(u) + EPS_W, CLAMP_MIN, CLAMP_MAX)
        # No Ln/Softplus ACT table on gen3 -> Newton on exp(s) = 1 + e^u:
        # s0 = relu(u); s += (1+e^u)*e^-s - 1 (quadratic, 4 iters to fp32).
        # All ACT funcs used (Exp/Relu/Square/Copy) share one table.
        nc.vector.tensor_scalar_min(out=u_t[:], in0=u_t[:], scalar1=80.0)
        nc.scalar.activation(out=s2_t[:], in_=u_t[:], func=Act.Exp)
        nc.vector.tensor_scalar_add(out=s2_t[:], in0=s2_t[:], scalar1=1.0)
        nc.scalar.activation(out=w_t[:], in_=u_t[:], func=Act.Relu)
        for _ in range(4):
            nc.scalar.activation(out=s1_t[:], in_=w_t[:], func=Act.Exp, scale=-1.0)
            nc.vector.tensor_mul(out=s1_t[:], in0=s2_t[:], in1=s1_t[:])
            nc.vector.scalar_tensor_tensor(
                out=w_t[:], in0=s1_t[:], scalar=-1.0, in1=w_t[:],
                op0=Alu.add, op1=Alu.add,
            )
        nc.vector.tensor_scalar(
            out=w_t[:], in0=w_t[:], scalar1=EPS_W, scalar2=CLAMP_MIN,
            op0=Alu.add, op1=Alu.max,
        )
        nc.vector.tensor_scalar_min(out=w_t[:], in0=w_t[:], scalar1=CLAMP_MAX)

        # xk = x * (1 - unk) ;  b = -D(xk):  b[i] = xk[i] - xk[i+1]
        nc.gpsimd.tensor_mul(out=s1_t[:], in0=x_t[:], in1=unk_t[:])
        nc.vector.tensor_sub(out=x_t[:], in0=x_t[:], in1=s1_t[:])
        nc.vector.tensor_sub(
            out=b_t[:, 0:WF - 1], in0=x_t[:, 0:WF - 1], in1=x_t[:, 1:WF]
        )
        # rhs = unk * Dt(w2 * b):  tb2 = w*(w*b);  z0[j] = tb2[j-1]-tb2[j]
        nc.vector.tensor_mul(out=s2_t[:, 0:WF - 1], in0=w_t[:, 0:WF - 1],
                             in1=b_t[:, 0:WF - 1])
        nc.vector.tensor_mul(out=s1_t[:, 0:WF - 1], in0=w_t[:, 0:WF - 1],
                             in1=s2_t[:, 0:WF - 1])
        nc.vector.tensor_sub(
            out=s2_t[:, 1:WF - 1], in0=s1_t[:, 0:WF - 2], in1=s1_t[:, 1:WF - 1]
        )
        nc.gpsimd.tensor_mul(
            out=rc_t[:, 1:WF - 1], in0=unk_t[:, 1:WF - 1], in1=s2_t[:, 1:WF - 1]
        )
        nc.scalar.copy(out=p_a[:, 1:WF - 1], in_=rc_t[:, 1:WF - 1])

        # rs0 = per-row sum(rc^2) over core region
        ps = rowdot(rc_t[:, CORE])
        nc.vector.tensor_copy(out=rs_a[:], in_=ps[:])

        def split_tt(op, out_t, o_lo, o_hi, in0_t, i0_lo, in1_t, i1_lo):
            """Emit one logical tensor_tensor as a DVE part + a GPSIMD part,
            split at FSPLIT along the columns."""
            n = o_hi - o_lo
            cs = int(n * FSPLIT)
            for eng, lo, ln in ((nc.vector, 0, cs), (nc.gpsimd, cs, n - cs)):
                if ln <= 0:
                    continue
                eng.tensor_tensor(
                    out=out_t[:, o_lo + lo:o_lo + lo + ln],
                    in0=in0_t[:, i0_lo + lo:i0_lo + lo + ln],
                    in1=in1_t[:, i1_lo + lo:i1_lo + lo + ln],
                    op=op,
                )

        def bc0(sc_tile, ln):
            """[128,1] SBUF scalar broadcast along free dim via 0-stride AP."""
            ap = sc_tile[:]
            return bass.AP(ap.tensor, ap.offset, [list(ap.ap[0]), [0, ln]])

        def split_stt(out_t, o_lo, o_hi, in0_t, i0_lo, sc_sb, in1_t, i1_lo,
                      q_t):
            """out = in0*scalar + in1, split DVE stt / GPSIMD mul+add at FSTT.
            sc_sb: [128,1] SBUF scalar; q_t: scratch tile for the GPSIMD mul
            (its [o_lo+cs, o_hi) columns are clobbered)."""
            n = o_hi - o_lo
            cs = int(n * FSTT)
            if cs > 0:
                nc.vector.scalar_tensor_tensor(
                    out=out_t[:, o_lo:o_lo + cs], in0=in0_t[:, i0_lo:i0_lo + cs],
                    scalar=sc_sb[:], in1=in1_t[:, i1_lo:i1_lo + cs],
                    op0=Alu.mult, op1=Alu.add,
                )
            ln = n - cs
            if ln > 0:
                nc.gpsimd.tensor_tensor(
                    out=q_t[:, o_lo + cs:o_hi],
                    in0=in0_t[:, i0_lo + cs:i0_lo + n],
                    in1=bc0(sc_sb, ln), op=Alu.mult,
                )
                nc.gpsimd.tensor_add(
                    out=out_t[:, o_lo + cs:o_hi],
                    in0=q_t[:, o_lo + cs:o_hi],
                    in1=in1_t[:, i1_lo + cs:i1_lo + n],
                )

        # ---------- CG loop ----------
        # p ping-pongs between p_a/p_b; the pKp dot overlaps t/z; beta comes
        # from the CG recurrence rs' = rs - 2a*pKp + a^2*|Kp|^2 (exact-arith
        # identical) so the |Kp|^2 dot overlaps the rc update; the V update
        # is emitted last and fills scheduling gaps.  The four tensor_tensor
        # ops are column-split DVE/GPSIMD.
        for k in range(MAX_ITER):
            rs_cur = rs_a if k % 2 == 0 else rs_b
            rs_new = rs_b if k % 2 == 0 else rs_a
            p_cur = p_a if k % 2 == 0 else p_b
            p_nxt = p_b if k % 2 == 0 else p_a

            # tm = D p ; that = w * tm ; pKp = per-row sum(that^2)
            split_tt(Alu.subtract, s1_t, 0, WF - 1, p_cur, 1, p_cur, 0)
            split_tt(Alu.mult, s2_t, 0, WF - 1, w_t, 0, s1_t, 0)
            pkp_ps = rowdot(s2_t[:, CORE])

            # t = w * that ; z[j] = t[j-1] - t[j] ; Kp = unk * z
            # (independent of alpha -> overlaps the dot chain)
            split_tt(Alu.mult, s1_t, 0, WF - 1, w_t, 0, s2_t, 0)
            split_tt(Alu.subtract, s2_t, 1, WF - 1, s1_t, 0, s1_t, 1)
            split_tt(Alu.mult, u_t, 1, WF - 1, unk_t, 1, s2_t, 1)

            # alpha = rs / (pKp + 1e-30); broadcast; nabc = -alpha per partition
            nc.vector.tensor_scalar_add(out=den[:], in0=pkp_ps[:], scalar1=1e-30)
            nc.vector.reciprocal(out=den[:], in_=den[:])
            nc.vector.tensor_mul(out=alpha[:], in0=rs_cur[:], in1=den[:])
            abc = psum.tile([128, 1], f32, tag="ps_b")
            nc.tensor.matmul(abc[:], brows[:], alpha[:], start=True, stop=True)
            nc.vector.tensor_scalar_mul(out=nabc[:], in0=abc[:], scalar1=-1.0)
            nc.vector.tensor_copy(out=abc_sb[:], in_=abc[:])

            # |Kp|^2 dot (ACT+PE) overlaps the rc update
            kpkp_ps = rowdot(u_t[:, CORE])
            split_stt(rc_t, 1, WF - 1, u_t, 1, nabc, rc_t, 1, s1_t)

            # rs_new = rs - 2*alpha*pKp + alpha^2*KpKp ; beta = rs_new/(rs+eps)
            nc.vector.tensor_mul(out=t3s[:], in0=alpha[:], in1=pkp_ps[:])
            nc.vector.tensor_mul(out=aas[:], in0=alpha[:], in1=alpha[:])
            nc.vector.tensor_mul(out=aas[:], in0=aas[:], in1=kpkp_ps[:])
            nc.vector.scalar_tensor_tensor(
                out=t3s[:], in0=t3s[:], scalar=-2.0, in1=rs_cur[:],
                op0=Alu.mult, op1=Alu.add,
            )
            nc.vector.tensor_add(out=rs_new[:], in0=t3s[:], in1=aas[:])
            nc.vector.tensor_scalar_add(out=den[:], in0=rs_cur[:], scalar1=1e-30)
            nc.vector.reciprocal(out=den[:], in_=den[:])
            nc.vector.tensor_mul(out=beta[:], in0=rs_new[:], in1=den[:])
            bbc = psum.tile([128, 1], f32, tag="ps_b")
            nc.tensor.matmul(bbc[:], brows[:], beta[:], start=True, stop=True)
            nc.vector.tensor_copy(out=bbc_sb[:], in_=bbc[:])
            split_stt(p_nxt, 1, WF - 1, p_cur, 1, bbc_sb, rc_t, 1, s2_t)
            # V += alpha*p: emitted last so it fills gaps of the next
            # iteration (p_cur stays live thanks to the ping-pong).
            split_stt(V_t, 1, WF - 1, p_cur, 1, abc_sb, V_t, 1, u_t)

        # ---------- postamble ----------
        # r = D V - b ; phi = per-row sum((w*r)^2) ; outputs
        nc.vector.tensor_sub(
            out=s1_t[:, 0:WF - 1], in0=V_t[:, 1:WF], in1=V_t[:, 0:WF - 1]
        )
        nc.vector.tensor_sub(
            out=s2_t[:, 0:WF - 1], in0=s1_t[:, 0:WF - 1], in1=b_t[:, 0:WF - 1]
        )
        nc.vector.tensor_mul(
            out=s1_t[:, 0:WF - 1], in0=w_t[:, 0:WF - 1], in1=s2_t[:, 0:WF - 1]
        )
        phi_ps = rowdot(s1_t[:, CORE])
        nc.vector.tensor_copy(out=phi_t[:], in_=phi_ps[:])

        for r in range(ROWS):
            nc.sync.dma_start(
                out=bass.AP(v_out, r * N, [[CH, PPR], [1, CH]]),
                in_=V_t[r * PPR:(r + 1) * PPR, CORE],
            )
            nc.sync.dma_start(
                out=bass.AP(r_out, r * N, [[CH, PPR], [1, CH]]),
                in_=s2_t[r * PPR:(r + 1) * PPR, CORE],
            )
            nc.sync.dma_start(
                out=bass.AP(w_out, r * N, [[CH, PPR], [1, CH]]),
                in_=w_t[r * PPR:(r + 1) * PPR, CORE],
            )
        nc.sync.dma_start(
            out=bass.AP(phi_out, 0, [[1, ROWS], [1, 1]]), in_=phi_t[:]
        )

        loop_cm.__exit__(None, None, None)

    nc.compile()
    return nc


def get_program(debug=False, enable_asserts=False, repeat=1):
    key = (debug, enable_asserts, repeat)
    if key not in _CACHE:
        _CACHE[key] = _build_program(
            debug=debug, enable_asserts=enable_asserts, repeat=repeat
        )
    return _CACHE[key]


def make_in_maps(u_logits, x_full, unknown_idx):
    u = np.ascontiguousarray(np.asarray(u_logits, dtype=np.float32))
    x = np.ascontiguousarray(np.asarray(x_full, dtype=np.float32))
    idx = np.asarray(unknown_idx).astype(np.int64)

    unk = np.zeros(PADN, np.float32)
    unk[H + idx] = 1.0
    u_pad = np.full((B, PADN), PAD_U, np.float32)
    u_pad[:, H:H + M] = u
    x_pad = np.zeros((B, PADN), np.float32)
    x_pad[:, H:H + N] = x

    in_maps = []
    for c in range(NCORES):
        in_maps.append({
            "u_in": u_pad[ROWS * c: ROWS * (c + 1)],
            "x_in": x_pad[ROWS * c: ROWS * (c + 1)],
            "unk_in": unk,
        })
    return in_maps, idx


def assemble_outputs(results, idx):
    phi = np.concatenate([res["phi_out"].reshape(ROWS) for res in results])
    V = np.concatenate([res["v_out"] for res in results], axis=0)
    r = np.concatenate([res["r_out"] for res in results], axis=0)[:, :M]
    w = np.concatenate([res["w_out"] for res in results], axis=0)[:, :M]
    v = V[:, idx]
    return (
        np.ascontiguousarray(phi.astype(np.float32)),
        np.ascontiguousarray(v.astype(np.float32)),
        np.ascontiguousarray(r.astype(np.float32)),
        np.ascontiguousarray(w.astype(np.float32)),
    )


def kernel(u_logits, x_full, unknown_idx):
    from concourse.bass_utils import run_bass_kernel_spmd

    nc = get_program()
    in_maps, idx = make_in_maps(u_logits, x_full, unknown_idx)
    res = run_bass_kernel_spmd(nc, in_maps, list(range(NCORES)))
    return assemble_outputs(res.results, idx)


if __name__ == "__main__":
    rng = np.random.default_rng(0)
    u = rng.standard_normal((B, M), np.float32)
    x = rng.standard_normal((B, N), np.float32)
    idx = np.sort(rng.permutation(N)[:NMISS])
    outs = kernel(u_logits=u, x_full=x, unknown_idx=idx)
    for name, o in zip(["phi", "v", "r", "w"], outs):
        print(name, o.shape, o.dtype, float(np.abs(o).max()))
